# revision 12
# baseline (speedup 1.0000x reference)
# BitLinear 1.58 (ternary-weight linear with int8 activation quantization)
# on 8 Trainium2 NeuronCores via Bass/Tile.
#
# Reference computation (fp32):
#   w_scale = max(mean(|W|), 1e-5)           (global over the full weight)
#   W_q     = clip(round(W / w_scale), -1, 1)          (ternary)
#   gamma   = max(max(|x|), 1e-5)            (global over the full activation)
#   x_q     = clip(round(x * 128/gamma), -128, 127)
#   out     = (x_q @ W_q^T) * (gamma*w_scale/128) + bias
#
# Sharding: data-parallel over the 8192 tokens (1024 tokens/core), weight
# replicated. Global scales via one 8-byte AllGather of per-core partials
# (local absmax(x shard), local sum|W-slice| over a distinct 1/8 of W).
#
# Schedule (v5, from v1-v4 traces):
#  - The critical path is: 24 MiB of stats reads (x 16 + w-slice 8, HBM
#    ~330 GB/s with both HWDGE rings) -> 8-byte AllGather (~35us) ->
#    quantize ramp -> 2048 bf16 matmuls (442us floor at N=512). Splitting
#    the AllGather in two (v4) loses: consecutive collectives serialize on
#    the ncfw stream, and anything sharing the gpsimd queue blocks on the
#    first collective's completion-wait.
#  - bias_rep broadcast stages through bias_rep row 0 (DMA + 8 K=1 PE
#    matmuls, no tile-pool interaction) so the PE queue drains right away
#    and the runtime barrier preceding the collective clears early.
#  - Stats reads get the HBM exclusively: main-loop W/x DMAs are dep-gated
#    behind the last stats chunks, then flood the AllGather wait window.
#    w-slice uses the v1 16-chunk "(a p x) y" view bit-exactly (the fp32
#    partial-sum order sets w_scale's last ulp; a 1-ulp shift flips
#    boundary weights at ~100x max-err cost). x uses 32 flat [128,1024]
#    chunks (max is order-independent).
#  - Ternarize: batched [128,2048] tiles (4 k-slices), 3 fused DVE ops
#    (~3.2us/tile vs PE consumption 3.46us/tile; ACT is ~3x slower and
#    starves the PE into HAM-cold oscillation if put on this path).
#  - x-quantize pass1 alternates ACT/DVE per k-tile; pass2 on DVE. xin
#    re-reads prefetch deep (9 tiles) during the AllGather wait.
#  - Token-halved PSUM rotation (4 accumulating + 4 evicting banks), one
#    evict per 8 k-steps: no column-boundary PE bursts.
#
# Quantized operands in bf16 (exact: x_q in [-128,127], W_q in {-1,0,1},
# PSUM accumulates fp32, sums bounded by 4096*128 = 2^19 < 2^24).
# Rounding: round-half-even in fp32 via the magic constant
# (v + 1.5*2^23) - 1.5*2^23, fused into tensor_scalar/activation ops.

import numpy as np
from contextlib import ExitStack

import concourse.bass as bass
import concourse.tile as tile
from concourse import bacc, mybir
from concourse import bass_utils

N_CORES = 8
IN_F = 4096
OUT_F = 4096
TOKENS = 8192  # 4 * 2048
TPC = TOKENS // N_CORES  # tokens per core = 1024
OSL = OUT_F // N_CORES  # per-core weight-stats slice = 512 out_features

KT = IN_F // 128  # 32 k-tiles
KG = KT // 4  # 8 k-groups of 4 (ternarize batch)
CT = OUT_F // 512  # 8 of-columns
TT = TPC // 128  # 8 token-tiles (two halves of 4)

MAGIC = 12582912.0  # 1.5 * 2**23: (v + MAGIC) - MAGIC == round-half-even(v)
EPS = 1e-5
F32 = mybir.dt.float32
BF16 = mybir.dt.bfloat16

NXC = 32  # x-stats chunks [128, 1024]
NWC = 16  # w-stats chunks [128, 1024] (v1 chunking, keeps w_scale bit-exact)

_cache = {}


def _build(dbg=False):
    nc = bacc.Bacc("TRN2", target_bir_lowering=False, debug=False, num_devices=N_CORES)
    xT = nc.dram_tensor("xT", [IN_F, TPC], F32, kind="ExternalInput").ap()
    wT = nc.dram_tensor("wT", [IN_F, OUT_F], F32, kind="ExternalInput").ap()
    wS = nc.dram_tensor("wS", [IN_F, OSL], F32, kind="ExternalInput").ap()
    bias = nc.dram_tensor("bias", [OUT_F], F32, kind="ExternalInput").ap()
    out = nc.dram_tensor("out", [TPC, OUT_F], F32, kind="ExternalOutput").ap()
    if dbg:
        dbg_t = nc.dram_tensor("dbg", [16], F32, kind="ExternalOutput").ap()

    with tile.TileContext(nc) as tc, ExitStack() as ctx:
        ep = ctx.enter_context
        singles = ep(tc.tile_pool(name="singles", bufs=1))
        stream_pool = ep(tc.tile_pool(name="stream", bufs=6))
        win_pool = ep(tc.tile_pool(name="win", bufs=3))
        xin_pool = ep(tc.tile_pool(name="xin", bufs=9))
        xq_pool = ep(tc.tile_pool(name="xq", bufs=KT))
        wq_pool = ep(tc.tile_pool(name="wq", bufs=9))
        ost_pool = ep(tc.tile_pool(name="ost", bufs=2))
        psum_pool = ep(tc.tile_pool(name="psum", bufs=8, space="PSUM"))
        dram = ep(tc.tile_pool(name="dram", bufs=1, space="DRAM"))

        ones_row = singles.tile([1, 128], F32)  # for partition-broadcast matmul
        nc.vector.memset(ones_row[:], 1.0)

        # ---- bias replicated across partitions, FIRST (PE queue drains
        # immediately -> pre-collective barrier clears early). Stages via
        # bias_rep's own row 0: the K=1 matmul reads row 0 of a slice, the
        # copy then overwrites the full slice (row 0 keeps its value).
        bias_rep = singles.tile([128, OUT_F], F32)
        nc.sync.dma_start(bias_rep[0:1, :], bias[:])
        for n in range(CT):
            bp = psum_pool.tile([128, 512], F32, tag="ps", name=f"biasps{n}")
            nc.tensor.matmul(
                bp[:], ones_row[:], bias_rep[0:1, n * 512 : (n + 1) * 512],
                start=True, stop=True,
            )
            nc.scalar.copy(bias_rep[:, n * 512 : (n + 1) * 512], bp[:])

        # ---- stats reads: exclusive use of both HWDGE rings ----
        xv = xT[:].rearrange("(p x) y -> p (x y)", p=128)
        wv = wS[:].rearrange("(a p x) y -> a p (x y)", p=128, x=2)

        xm = singles.tile([128, NXC], F32)
        wm = singles.tile([128, NWC], F32)
        XC = IN_F * TPC // 128 // NXC  # 1024
        last_stats = {}
        for j in range(NWC):
            st = stream_pool.tile([128, 1024], F32, tag="stream", name=f"sw{j}")
            eng = nc.sync if j % 2 == 0 else nc.scalar
            last_stats[j % 2] = eng.dma_start(st[:], wv[j])
            nc.scalar.activation(
                st[:], st[:], mybir.ActivationFunctionType.Abs,
                accum_out=wm[:, j : j + 1],
            )
        # w fold early (gpsimd transpose queued ahead of the x-side's)
        wsumc = singles.tile([128, 1], F32)
        nc.vector.tensor_reduce(
            wsumc[:], wm[:], axis=mybir.AxisListType.X, op=mybir.AluOpType.add
        )
        wsumT = singles.tile([1, 128], F32)
        nc.gpsimd.dma_start(wsumT[:], wsumc[:])
        wsum = singles.tile([1, 1], F32)
        nc.vector.tensor_reduce(
            wsum[:], wsumT[:], axis=mybir.AxisListType.X, op=mybir.AluOpType.add
        )

        for j in range(NXC):
            st = stream_pool.tile([128, XC], F32, tag="stream", name=f"sx{j}")
            eng = nc.sync if j % 2 == 0 else nc.scalar
            last_stats[j % 2] = eng.dma_start(st[:], xv[:, j * XC : (j + 1) * XC])
            nc.vector.tensor_reduce(
                xm[:, j : j + 1], st[:], axis=mybir.AxisListType.X,
                op=mybir.AluOpType.max, apply_absolute_value=True,
            )
        xmax = singles.tile([128, 1], F32)
        nc.vector.tensor_reduce(
            xmax[:], xm[:], axis=mybir.AxisListType.X, op=mybir.AluOpType.max
        )
        xmaxT = singles.tile([1, 128], F32)
        nc.gpsimd.dma_start(xmaxT[:], xmax[:])
        gx = singles.tile([1, 1], F32)
        nc.vector.tensor_reduce(
            gx[:], xmaxT[:], axis=mybir.AxisListType.X, op=mybir.AluOpType.max
        )

        # ---- share both partial stats: one 8-byte-per-core AllGather ----
        cc_sb = singles.tile([1, 2], F32)
        nc.vector.tensor_copy(cc_sb[0:1, 0:1], gx[:])
        nc.vector.tensor_copy(cc_sb[0:1, 1:2], wsum[:])
        cc_in = dram.tile([2], F32)
        cc_out = dram.tile([2 * N_CORES], F32)
        nc.gpsimd.dma_start(cc_in[:], cc_sb[:])
        nc.gpsimd.collective_compute(
            "AllGather", mybir.AluOpType.bypass,
            replica_groups=[list(range(N_CORES))],
            ins=[cc_in.opt()], outs=[cc_out.opt()],
        )
        g16 = singles.tile([1, 2 * N_CORES], F32)
        nc.gpsimd.dma_start(g16[:], cc_out[:])
        g3 = g16[:].rearrange("p (r two) -> p two r", two=2)

        # ---- combine gathered stats; per-partition scalar math ----
        gsum = singles.tile([1, 1], F32)
        nc.vector.tensor_reduce(
            gsum[:], g3[0:1, 1:2, :], axis=mybir.AxisListType.X,
            op=mybir.AluOpType.add,
        )
        wscale = singles.tile([1, 1], F32)
        nc.vector.tensor_scalar(
            wscale[:], gsum[:], 1.0 / (OUT_F * IN_F), EPS,
            mybir.AluOpType.mult, mybir.AluOpType.max,
        )
        gmax = singles.tile([1, 1], F32)
        nc.vector.tensor_reduce(
            gmax[:], g3[0:1, 0:1, :], axis=mybir.AxisListType.X,
            op=mybir.AluOpType.max,
        )
        gamma = singles.tile([1, 1], F32)
        nc.vector.tensor_scalar(gamma[:], gmax[:], EPS, None, mybir.AluOpType.max)

        def newton_recip(name, src):
            # correctly-rounded-ish 1/src: HW reciprocal + one Newton step
            r0 = singles.tile([1, 1], F32, tag=f"{name}r0")
            nc.vector.reciprocal(r0[:], src[:])
            t = singles.tile([1, 1], F32, tag=f"{name}t")
            nc.vector.tensor_tensor(t[:], src[:], r0[:], op=mybir.AluOpType.mult)
            u = singles.tile([1, 1], F32, tag=f"{name}u")
            nc.vector.tensor_scalar(
                u[:], t[:], -1.0, 2.0, mybir.AluOpType.mult, mybir.AluOpType.add
            )
            r1 = singles.tile([1, 1], F32, tag=f"{name}r1")
            nc.vector.tensor_tensor(r1[:], r0[:], u[:], op=mybir.AluOpType.mult)
            return r1

        rw = newton_recip("rw", wscale)  # 1/w_scale
        rg = newton_recip("rg", gamma)   # 1/gamma
        pack3 = singles.tile([1, 3], F32)
        nc.vector.tensor_scalar(
            pack3[0:1, 0:1], rg[:], 128.0, None, mybir.AluOpType.mult
        )
        nc.vector.tensor_copy(pack3[0:1, 1:2], rw[:])
        gws = singles.tile([1, 1], F32)
        nc.vector.tensor_tensor(gws[:], gamma[:], wscale[:], op=mybir.AluOpType.mult)
        nc.vector.tensor_scalar(
            pack3[0:1, 2:3], gws[:], 2.0 ** -7, None, mybir.AluOpType.mult
        )
        # broadcast [s_x, r_w, s_o] to all partitions via a K=1 PE matmul
        bp3 = psum_pool.tile([128, 3], F32, tag="ps", name="bp3")
        nc.tensor.matmul(bp3[:], ones_row[:], pack3[:], start=True, stop=True)
        b3 = singles.tile([128, 3], F32)
        nc.vector.tensor_copy(b3[:], bp3[:])
        s_x = b3[:, 0:1]
        r_w = b3[:, 1:2]
        s_o = b3[:, 2:3]

        if dbg:
            dsb = singles.tile([1, 16], F32)
            nc.vector.memset(dsb[:], 0.0)
            nc.vector.tensor_copy(dsb[0:1, 0:1], gamma[:])
            nc.vector.tensor_copy(dsb[0:1, 1:2], wscale[:])
            nc.vector.tensor_copy(dsb[0:1, 2:5], b3[96:97, :])
            nc.sync.dma_start(dbg_t[:], dsb[:])

        # ---- main loop ----
        xq = [None] * KT

        def emit_xq(k):
            # x requantize read; both rings, first ones gated behind stats
            xin = xin_pool.tile([128, TPC], F32, tag="xin", name=f"xin_q{k}")
            eng = nc.sync if k % 2 == 0 else nc.scalar
            xin_dma = eng.dma_start(xin[:], xT[k * 128 : (k + 1) * 128, :])
            if k < 9:
                for ring in last_stats:
                    tile.add_dep_helper(
                        xin_dma.ins, last_stats[ring].ins, sync=True,
                        reason="hold x re-read until stats reads finish",
                    )
            # pass1: t = x*s_x + MAGIC (rounds to int); alternate ACT/DVE per
            # k to halve the post-gamma ramp. pass2 (DVE): min(t-M, 127)
            if k % 2 == 0:
                nc.scalar.activation(
                    xin[:], xin[:], mybir.ActivationFunctionType.Copy,
                    scale=s_x, bias=MAGIC,
                )
            else:
                nc.vector.tensor_scalar(
                    xin[:], xin[:], s_x, MAGIC, mybir.AluOpType.mult,
                    mybir.AluOpType.add,
                )
            xq_k = xq_pool.tile([128, TPC], BF16, tag="xq", name=f"xq{k}")
            nc.vector.tensor_scalar(
                xq_k[:], xin[:], MAGIC, 127.0, mybir.AluOpType.subtract,
                mybir.AluOpType.min,
            )
            xq[k] = xq_k

        def emit_wq(c, g):
            # one DMA brings 4 k-slices [128, 2048]; 3 fused DVE passes:
            # t = w*r_w + MAGIC; clip to MAGIC+-1; -MAGIC (cast bf16)
            win = win_pool.tile([128, 2048], F32, tag="win", name=f"win_c{c}_g{g}")
            src = wT[g * 512 : (g + 1) * 512, c * 512 : (c + 1) * 512]
            eng = nc.sync if g % 2 == 0 else nc.scalar
            win_dma = eng.dma_start(
                win[:].rearrange("p (x y) -> p x y", y=512),
                src.rearrange("(x p) y -> p x y", p=128),
            )
            if c == 0 and g < 3:
                for ring in last_stats:
                    tile.add_dep_helper(
                        win_dma.ins, last_stats[ring].ins, sync=True,
                        reason="hold weight prefetch until stats reads finish",
                    )
            nc.vector.tensor_scalar(
                win[:], win[:], r_w, MAGIC, mybir.AluOpType.mult,
                mybir.AluOpType.add,
            )
            nc.vector.tensor_scalar(
                win[:], win[:], MAGIC + 1.0, MAGIC - 1.0, mybir.AluOpType.min,
                mybir.AluOpType.max,
            )
            wq = wq_pool.tile([128, 2048], BF16, tag="wq", name=f"wq_c{c}_g{g}")
            nc.vector.tensor_scalar(
                wq[:], win[:], MAGIC, None, mybir.AluOpType.subtract
            )
            return wq

        def emit_evict(c, t, psum_t):
            of = c * 512
            osb = ost_pool.tile([128, 512], F32, tag="ost", name=f"osb_c{c}_t{t}")
            # out = psum * s_o + bias, one DVE op straight from PSUM
            nc.vector.scalar_tensor_tensor(
                osb[:], psum_t[:], s_o, bias_rep[:, of : of + 512],
                op0=mybir.AluOpType.mult, op1=mybir.AluOpType.add,
            )
            eng = nc.scalar if t % 2 == 0 else nc.sync
            eng.dma_start(out[t * 128 : (t + 1) * 128, of : of + 512], osb[:])

        prev = None  # (c, half_t0, psums) awaiting evict
        for c in range(CT):
            wqs = [None] * KG
            for half in range(2):
                t0 = half * 4
                psums = [
                    psum_pool.tile(
                        [128, 512], F32, tag="ps", name=f"ps_c{c}_t{t0 + i}"
                    )
                    for i in range(4)
                ]
                for k in range(KT):
                    if c == 0 and half == 0:
                        emit_xq(k)
                    if half == 0 and k % 4 == 0:
                        wqs[k // 4] = emit_wq(c, k // 4)
                    # previous half's evicts, one per 8 k-steps: banks free
                    # gradually without a DVE burst
                    if prev is not None and k % 8 == 4:
                        pc, pt0, pp = prev
                        i = (k - 4) // 8
                        emit_evict(pc, pt0 + i, pp[i])
                    wq_s = wqs[k // 4][:, (k % 4) * 512 : (k % 4 + 1) * 512]
                    for i in range(4):
                        t = t0 + i
                        nc.tensor.matmul(
                            psums[i][:], xq[k][:, t * 128 : (t + 1) * 128], wq_s,
                            start=(k == 0), stop=(k == KT - 1),
                        )
                prev = (c, t0, psums)
        pc, pt0, pp = prev
        for i in range(4):
            emit_evict(pc, pt0 + i, pp[i])

    nc.compile()
    return nc


def _prep_inputs(x, weight, bias):
    x2 = np.ascontiguousarray(x.reshape(TOKENS, IN_F).T)  # [IN_F, TOKENS]
    wT = np.ascontiguousarray(weight.T)  # [IN_F, OUT_F]
    in_maps = []
    for i in range(N_CORES):
        in_maps.append(
            {
                "xT": np.ascontiguousarray(x2[:, i * TPC : (i + 1) * TPC]),
                "wT": wT,
                "wS": np.ascontiguousarray(wT[:, i * OSL : (i + 1) * OSL]),
                "bias": bias,
            }
        )
    return in_maps


def _run(x, weight, bias, trace=False):
    if "nc" not in _cache:
        _cache["nc"] = _build()
    nc = _cache["nc"]
    in_maps = _prep_inputs(
        np.asarray(x, dtype=np.float32),
        np.asarray(weight, dtype=np.float32),
        np.asarray(bias, dtype=np.float32),
    )
    res = bass_utils.run_bass_kernel_spmd(
        nc, in_maps, list(range(N_CORES)), trace=trace
    )
    full = np.concatenate(
        [res.results[i]["out"] for i in range(N_CORES)], axis=0
    )
    return full.reshape(4, 2048, OUT_F), res


def kernel(x, weight, bias):
    out, _ = _run(x, weight, bias)
    return out


# revision 13
# speedup vs baseline: 1.1101x; 1.1101x over previous
# BitLinear 1.58 (ternary-weight linear with int8 activation quantization)
# on 8 Trainium2 NeuronCores via Bass/Tile.
#
# Reference computation (fp32):
#   w_scale = max(mean(|W|), 1e-5)           (global over the full weight)
#   W_q     = clip(round(W / w_scale), -1, 1)          (ternary)
#   gamma   = max(max(|x|), 1e-5)            (global over the full activation)
#   x_q     = clip(round(x * 128/gamma), -128, 127)
#   out     = (x_q @ W_q^T) * (gamma*w_scale/128) + bias
#
# Sharding: data-parallel over the 8192 tokens (1024 tokens/core), weight
# replicated. Global scales via one 8-byte AllGather of per-core partials
# (local absmax(x shard), local sum|W-slice| over a distinct 1/8 of W).
#
# Schedule (v5, from v1-v4 traces):
#  - The critical path is: 24 MiB of stats reads (x 16 + w-slice 8, HBM
#    ~330 GB/s with both HWDGE rings) -> 8-byte AllGather (~35us) ->
#    quantize ramp -> 2048 bf16 matmuls (442us floor at N=512). Splitting
#    the AllGather in two (v4) loses: consecutive collectives serialize on
#    the ncfw stream, and anything sharing the gpsimd queue blocks on the
#    first collective's completion-wait.
#  - bias_rep broadcast stages through bias_rep row 0 (DMA + 8 K=1 PE
#    matmuls, no tile-pool interaction) so the PE queue drains right away
#    and the runtime barrier preceding the collective clears early.
#  - Stats reads get the HBM exclusively: main-loop W/x DMAs are dep-gated
#    behind the last stats chunks, then flood the AllGather wait window.
#    w-slice uses the v1 16-chunk "(a p x) y" view bit-exactly (the fp32
#    partial-sum order sets w_scale's last ulp; a 1-ulp shift flips
#    boundary weights at ~100x max-err cost). x uses 32 flat [128,1024]
#    chunks (max is order-independent).
#  - Ternarize: batched [128,2048] tiles (4 k-slices), 3 fused DVE ops
#    (~3.2us/tile vs PE consumption 3.46us/tile; ACT is ~3x slower and
#    starves the PE into HAM-cold oscillation if put on this path).
#  - x-quantize pass1 alternates ACT/DVE per k-tile; pass2 on DVE. xin
#    re-reads prefetch deep (9 tiles) during the AllGather wait.
#  - Token-halved PSUM rotation (4 accumulating + 4 evicting banks), one
#    evict per 8 k-steps: no column-boundary PE bursts.
#
# Quantized operands in bf16 (exact: x_q in [-128,127], W_q in {-1,0,1},
# PSUM accumulates fp32, sums bounded by 4096*128 = 2^19 < 2^24).
# Rounding: round-half-even in fp32 via the magic constant
# (v + 1.5*2^23) - 1.5*2^23, fused into tensor_scalar/activation ops.

import numpy as np
from contextlib import ExitStack

import concourse.bass as bass
import concourse.tile as tile
from concourse import bacc, mybir
from concourse import bass_utils

N_CORES = 8
IN_F = 4096
OUT_F = 4096
TOKENS = 8192  # 4 * 2048
TPC = TOKENS // N_CORES  # tokens per core = 1024
OSL = OUT_F // N_CORES  # per-core weight-stats slice = 512 out_features

KT = IN_F // 128  # 32 k-tiles
KG = KT // 4  # 8 k-groups of 4 (ternarize batch)
CT = OUT_F // 512  # 8 of-columns
TT = TPC // 128  # 8 token-tiles (two halves of 4)

MAGIC = 12582912.0  # 1.5 * 2**23: (v + MAGIC) - MAGIC == round-half-even(v)
EPS = 1e-5
F32 = mybir.dt.float32
BF16 = mybir.dt.bfloat16

NXC = 32  # x-stats chunks [128, 1024]
NWC = 16  # w-stats chunks [128, 1024] (v1 chunking, keeps w_scale bit-exact)

_cache = {}


def _build(dbg=False):
    nc = bacc.Bacc("TRN2", target_bir_lowering=False, debug=False, num_devices=N_CORES)
    xT = nc.dram_tensor("xT", [IN_F, TPC], F32, kind="ExternalInput").ap()
    wT = nc.dram_tensor("wT", [IN_F, OUT_F], F32, kind="ExternalInput").ap()
    wS = nc.dram_tensor("wS", [IN_F, OSL], F32, kind="ExternalInput").ap()
    bias = nc.dram_tensor("bias", [OUT_F], F32, kind="ExternalInput").ap()
    out = nc.dram_tensor("out", [TPC, OUT_F], F32, kind="ExternalOutput").ap()
    if dbg:
        dbg_t = nc.dram_tensor("dbg", [16], F32, kind="ExternalOutput").ap()

    with tile.TileContext(nc) as tc, ExitStack() as ctx:
        ep = ctx.enter_context
        singles = ep(tc.tile_pool(name="singles", bufs=1))
        stream_pool = ep(tc.tile_pool(name="stream", bufs=6))
        win_pool = ep(tc.tile_pool(name="win", bufs=3))
        xin_pool = ep(tc.tile_pool(name="xin", bufs=9))
        xq_pool = ep(tc.tile_pool(name="xq", bufs=KT))
        wq_pool = ep(tc.tile_pool(name="wq", bufs=9))
        ost_pool = ep(tc.tile_pool(name="ost", bufs=2))
        psum_pool = ep(tc.tile_pool(name="psum", bufs=8, space="PSUM"))
        dram = ep(tc.tile_pool(name="dram", bufs=1, space="DRAM"))

        ones_row = singles.tile([1, 128], F32)  # for partition-broadcast matmul
        nc.vector.memset(ones_row[:], 1.0)

        # ---- bias replicated across partitions, FIRST (PE queue drains
        # immediately -> pre-collective barrier clears early). Stages via
        # bias_rep's own row 0: the K=1 matmul reads row 0 of a slice, the
        # copy then overwrites the full slice (row 0 keeps its value).
        bias_rep = singles.tile([128, OUT_F], F32)
        nc.sync.dma_start(bias_rep[0:1, :], bias[:])
        for n in range(CT):
            bp = psum_pool.tile([128, 512], F32, tag="ps", name=f"biasps{n}")
            nc.tensor.matmul(
                bp[:], ones_row[:], bias_rep[0:1, n * 512 : (n + 1) * 512],
                start=True, stop=True,
            )
            nc.vector.tensor_copy(bias_rep[:, n * 512 : (n + 1) * 512], bp[:])

        # ---- stats reads: exclusive use of both HWDGE rings ----
        xv = xT[:].rearrange("(p x) y -> p (x y)", p=128)
        wv = wS[:].rearrange("(a p x) y -> a p (x y)", p=128, x=2)

        xm = singles.tile([128, NXC], F32)
        wm = singles.tile([128, NWC], F32)
        XC = IN_F * TPC // 128 // NXC  # 1024
        last_stats = {}
        for j in range(NWC):
            st = stream_pool.tile([128, 1024], F32, tag="stream", name=f"sw{j}")
            eng = nc.sync if j % 2 == 0 else nc.scalar
            last_stats[j % 2] = eng.dma_start(st[:], wv[j])
            nc.scalar.activation(
                st[:], st[:], mybir.ActivationFunctionType.Abs,
                accum_out=wm[:, j : j + 1],
            )
        # w fold early (gpsimd transpose queued ahead of the x-side's)
        wsumc = singles.tile([128, 1], F32)
        nc.vector.tensor_reduce(
            wsumc[:], wm[:], axis=mybir.AxisListType.X, op=mybir.AluOpType.add
        )
        wsumT = singles.tile([1, 128], F32)
        nc.gpsimd.dma_start(wsumT[:], wsumc[:])
        wsum = singles.tile([1, 1], F32)
        nc.vector.tensor_reduce(
            wsum[:], wsumT[:], axis=mybir.AxisListType.X, op=mybir.AluOpType.add
        )

        for j in range(NXC):
            st = stream_pool.tile([128, XC], F32, tag="stream", name=f"sx{j}")
            eng = nc.sync if j % 2 == 0 else nc.scalar
            last_stats[j % 2] = eng.dma_start(st[:], xv[:, j * XC : (j + 1) * XC])
            nc.vector.tensor_reduce(
                xm[:, j : j + 1], st[:], axis=mybir.AxisListType.X,
                op=mybir.AluOpType.max, apply_absolute_value=True,
            )
        xmax = singles.tile([128, 1], F32)
        nc.vector.tensor_reduce(
            xmax[:], xm[:], axis=mybir.AxisListType.X, op=mybir.AluOpType.max
        )
        xmaxT = singles.tile([1, 128], F32)
        nc.gpsimd.dma_start(xmaxT[:], xmax[:])
        gx = singles.tile([1, 1], F32)
        nc.vector.tensor_reduce(
            gx[:], xmaxT[:], axis=mybir.AxisListType.X, op=mybir.AluOpType.max
        )

        # ---- share both partial stats: one 8-byte-per-core AllGather ----
        cc_sb = singles.tile([1, 2], F32)
        nc.vector.tensor_copy(cc_sb[0:1, 0:1], gx[:])
        nc.vector.tensor_copy(cc_sb[0:1, 1:2], wsum[:])
        cc_in = dram.tile([2], F32)
        cc_out = dram.tile([2 * N_CORES], F32)
        nc.gpsimd.dma_start(cc_in[:], cc_sb[:])
        nc.gpsimd.collective_compute(
            "AllGather", mybir.AluOpType.bypass,
            replica_groups=[list(range(N_CORES))],
            ins=[cc_in.opt()], outs=[cc_out.opt()],
        )
        g16 = singles.tile([1, 2 * N_CORES], F32)
        g16_dma = nc.gpsimd.dma_start(g16[:], cc_out[:])
        g3 = g16[:].rearrange("p (r two) -> p two r", two=2)

        # ---- PE pre-warm: ~3.5us of junk matmuls right when the AllGather
        # lands, so HAM is at K=8/8 when the real matmul stream starts ----
        junk_l = singles.tile([128, 128], BF16, tag="junkl")
        nc.vector.memset(junk_l[:], 0.0)
        junk_r = singles.tile([128, 512], BF16, tag="junkr")
        nc.vector.memset(junk_r[:], 0.0)
        jp = psum_pool.tile([128, 512], F32, tag="ps", name="junkps")
        for i in range(16):
            jmm = nc.tensor.matmul(jp[:], junk_l[:], junk_r[:], start=True, stop=True)
            if i == 0:
                tile.add_dep_helper(
                    jmm.ins, g16_dma.ins, sync=True,
                    reason="pre-warm PE as the gather result arrives",
                )

        # ---- combine gathered stats; per-partition scalar math ----
        gsum = singles.tile([1, 1], F32)
        nc.vector.tensor_reduce(
            gsum[:], g3[0:1, 1:2, :], axis=mybir.AxisListType.X,
            op=mybir.AluOpType.add,
        )
        wscale = singles.tile([1, 1], F32)
        nc.vector.tensor_scalar(
            wscale[:], gsum[:], 1.0 / (OUT_F * IN_F), EPS,
            mybir.AluOpType.mult, mybir.AluOpType.max,
        )
        gmax = singles.tile([1, 1], F32)
        nc.vector.tensor_reduce(
            gmax[:], g3[0:1, 0:1, :], axis=mybir.AxisListType.X,
            op=mybir.AluOpType.max,
        )
        gamma = singles.tile([1, 1], F32)
        nc.vector.tensor_scalar(gamma[:], gmax[:], EPS, None, mybir.AluOpType.max)

        def newton_recip(name, src):
            # correctly-rounded-ish 1/src: HW reciprocal + one Newton step
            r0 = singles.tile([1, 1], F32, tag=f"{name}r0")
            nc.vector.reciprocal(r0[:], src[:])
            t = singles.tile([1, 1], F32, tag=f"{name}t")
            nc.vector.tensor_tensor(t[:], src[:], r0[:], op=mybir.AluOpType.mult)
            u = singles.tile([1, 1], F32, tag=f"{name}u")
            nc.vector.tensor_scalar(
                u[:], t[:], -1.0, 2.0, mybir.AluOpType.mult, mybir.AluOpType.add
            )
            r1 = singles.tile([1, 1], F32, tag=f"{name}r1")
            nc.vector.tensor_tensor(r1[:], r0[:], u[:], op=mybir.AluOpType.mult)
            return r1

        rw = newton_recip("rw", wscale)  # 1/w_scale
        rg = newton_recip("rg", gamma)   # 1/gamma
        pack3 = singles.tile([1, 3], F32)
        nc.vector.tensor_scalar(
            pack3[0:1, 0:1], rg[:], 128.0, None, mybir.AluOpType.mult
        )
        nc.vector.tensor_copy(pack3[0:1, 1:2], rw[:])
        gws = singles.tile([1, 1], F32)
        nc.vector.tensor_tensor(gws[:], gamma[:], wscale[:], op=mybir.AluOpType.mult)
        nc.vector.tensor_scalar(
            pack3[0:1, 2:3], gws[:], 2.0 ** -7, None, mybir.AluOpType.mult
        )
        # broadcast [s_x, r_w, s_o] to all partitions via a K=1 PE matmul
        bp3 = psum_pool.tile([128, 3], F32, tag="ps", name="bp3")
        nc.tensor.matmul(bp3[:], ones_row[:], pack3[:], start=True, stop=True)
        b3 = singles.tile([128, 3], F32)
        nc.vector.tensor_copy(b3[:], bp3[:])
        s_x = b3[:, 0:1]
        r_w = b3[:, 1:2]
        s_o = b3[:, 2:3]

        if dbg:
            dsb = singles.tile([1, 16], F32)
            nc.vector.memset(dsb[:], 0.0)
            nc.vector.tensor_copy(dsb[0:1, 0:1], gamma[:])
            nc.vector.tensor_copy(dsb[0:1, 1:2], wscale[:])
            nc.vector.tensor_copy(dsb[0:1, 2:5], b3[96:97, :])
            nc.sync.dma_start(dbg_t[:], dsb[:])

        # ---- main loop ----
        xq = [None] * KT

        def emit_xq(k):
            # x requantize read; both rings, first ones gated behind stats
            xin = xin_pool.tile([128, TPC], F32, tag="xin", name=f"xin_q{k}")
            eng = nc.sync if k % 2 == 0 else nc.scalar
            xin_dma = eng.dma_start(xin[:], xT[k * 128 : (k + 1) * 128, :])
            if k < 9:
                for ring in last_stats:
                    tile.add_dep_helper(
                        xin_dma.ins, last_stats[ring].ins, sync=True,
                        reason="hold x re-read until stats reads finish",
                    )
            # pass1: t = x*s_x + MAGIC (rounds to int); alternate ACT/DVE per
            # k to halve the post-gamma ramp. pass2 (DVE): min(t-M, 127)
            if k % 2 == 0:
                nc.scalar.activation(
                    xin[:], xin[:], mybir.ActivationFunctionType.Copy,
                    scale=s_x, bias=MAGIC,
                )
            else:
                nc.vector.tensor_scalar(
                    xin[:], xin[:], s_x, MAGIC, mybir.AluOpType.mult,
                    mybir.AluOpType.add,
                )
            xq_k = xq_pool.tile([128, TPC], BF16, tag="xq", name=f"xq{k}")
            nc.vector.tensor_scalar(
                xq_k[:], xin[:], MAGIC, 127.0, mybir.AluOpType.subtract,
                mybir.AluOpType.min,
            )
            xq[k] = xq_k

        def emit_wq(c, g):
            # one DMA brings 4 k-slices [128, 2048]; 3 fused DVE passes:
            # t = w*r_w + MAGIC; clip to MAGIC+-1; -MAGIC (cast bf16)
            win = win_pool.tile([128, 2048], F32, tag="win", name=f"win_c{c}_g{g}")
            src = wT[g * 512 : (g + 1) * 512, c * 512 : (c + 1) * 512]
            eng = nc.sync if g % 2 == 0 else nc.scalar
            win_dma = eng.dma_start(
                win[:].rearrange("p (x y) -> p x y", y=512),
                src.rearrange("(x p) y -> p x y", p=128),
            )
            if c == 0 and g < 3:
                for ring in last_stats:
                    tile.add_dep_helper(
                        win_dma.ins, last_stats[ring].ins, sync=True,
                        reason="hold weight prefetch until stats reads finish",
                    )
            nc.vector.tensor_scalar(
                win[:], win[:], r_w, MAGIC, mybir.AluOpType.mult,
                mybir.AluOpType.add,
            )
            nc.vector.tensor_scalar(
                win[:], win[:], MAGIC + 1.0, MAGIC - 1.0, mybir.AluOpType.min,
                mybir.AluOpType.max,
            )
            wq = wq_pool.tile([128, 2048], BF16, tag="wq", name=f"wq_c{c}_g{g}")
            nc.vector.tensor_scalar(
                wq[:], win[:], MAGIC, None, mybir.AluOpType.subtract
            )
            return wq

        def emit_evict(c, t, psum_t):
            of = c * 512
            osb = ost_pool.tile([128, 512], F32, tag="ost", name=f"osb_c{c}_t{t}")
            # out = psum * s_o + bias, one DVE op straight from PSUM
            nc.vector.scalar_tensor_tensor(
                osb[:], psum_t[:], s_o, bias_rep[:, of : of + 512],
                op0=mybir.AluOpType.mult, op1=mybir.AluOpType.add,
            )
            eng = nc.scalar if t % 2 == 0 else nc.sync
            eng.dma_start(out[t * 128 : (t + 1) * 128, of : of + 512], osb[:])

        prev = None  # (c, half_t0, psums) awaiting evict
        for c in range(CT):
            wqs = [None] * KG
            for half in range(2):
                t0 = half * 4
                psums = [
                    psum_pool.tile(
                        [128, 512], F32, tag="ps", name=f"ps_c{c}_t{t0 + i}"
                    )
                    for i in range(4)
                ]
                for k in range(KT):
                    if c == 0 and half == 0:
                        emit_xq(k)
                    if half == 0 and k % 4 == 0:
                        wqs[k // 4] = emit_wq(c, k // 4)
                    # previous half's evicts, one per 8 k-steps: banks free
                    # gradually without a DVE burst
                    if prev is not None and k % 8 == 4:
                        pc, pt0, pp = prev
                        i = (k - 4) // 8
                        emit_evict(pc, pt0 + i, pp[i])
                    wq_s = wqs[k // 4][:, (k % 4) * 512 : (k % 4 + 1) * 512]
                    for i in range(4):
                        t = t0 + i
                        nc.tensor.matmul(
                            psums[i][:], xq[k][:, t * 128 : (t + 1) * 128], wq_s,
                            start=(k == 0), stop=(k == KT - 1),
                        )
                prev = (c, t0, psums)
        pc, pt0, pp = prev
        for i in range(4):
            emit_evict(pc, pt0 + i, pp[i])

    nc.compile()
    return nc


def _prep_inputs(x, weight, bias):
    x2 = np.ascontiguousarray(x.reshape(TOKENS, IN_F).T)  # [IN_F, TOKENS]
    wT = np.ascontiguousarray(weight.T)  # [IN_F, OUT_F]
    in_maps = []
    for i in range(N_CORES):
        in_maps.append(
            {
                "xT": np.ascontiguousarray(x2[:, i * TPC : (i + 1) * TPC]),
                "wT": wT,
                "wS": np.ascontiguousarray(wT[:, i * OSL : (i + 1) * OSL]),
                "bias": bias,
            }
        )
    return in_maps


def _run(x, weight, bias, trace=False):
    if "nc" not in _cache:
        _cache["nc"] = _build()
    nc = _cache["nc"]
    in_maps = _prep_inputs(
        np.asarray(x, dtype=np.float32),
        np.asarray(weight, dtype=np.float32),
        np.asarray(bias, dtype=np.float32),
    )
    res = bass_utils.run_bass_kernel_spmd(
        nc, in_maps, list(range(N_CORES)), trace=trace
    )
    full = np.concatenate(
        [res.results[i]["out"] for i in range(N_CORES)], axis=0
    )
    return full.reshape(4, 2048, OUT_F), res


def kernel(x, weight, bias):
    out, _ = _run(x, weight, bias)
    return out
